# revision 40
# baseline (speedup 1.0000x reference)
"""Trainium2 Bass kernel for a 2-layer GatedGraphConv encoder (9 convs, 18
message-passing + GRU steps) on N=50000 nodes, E=800000 edges, C=128.

Strategy (8 NeuronCores, SPMD single program):
  - Nodes are block-sharded: core c owns dst rows [c*6250, (c+1)*6250).
  - Edges live on the core that owns their dst.  Per core, edges are bucketed
    into 32-dst windows and padded to 128-edge chunks (chunk counts are the
    max over cores so the shared program works for every core).
  - Per layer: each core computes its m = x @ W slice with the tensor engine,
    an AllGather materializes the full message table in DRAM, dma_gather
    pulls the per-edge message rows into SBUF, and one 128x32 matmul per
    chunk (selector = ew-scaled one-hot of dst-in-window) accumulates
    agg^T = sum_e ew_e * m[src_e] directly in PSUM, feature-major.
  - The GRU update runs entirely feature-major: gate matmuls stream the
    512-node PSUM group through pre-transposed GRU weights; sigmoids/tanh on
    the scalar engine (per-partition bias = per-channel bias), elementwise on
    the vector engine.  x^T stays resident in SBUF across all 18 layers.
"""
import numpy as np

import concourse.bacc as bacc
import concourse.mybir as mybir
import concourse.tile as tile
from concourse import bass_utils

N = 50000
C = 128
NCORES = 8
NPC = N // NCORES            # 6250
WIN = 64                     # dst nodes per selector window
GRP = 8                      # windows per 512-col PSUM group
CHUNK = 128                  # edges per selector matmul
# Two INDEPENDENT half-AllGathers per layer into m_full [50000, 128] (HW
# overlaps independent collectives ~2x vs chained).  Core c's local nodes
# [0,3125) land at rows c*3125+local, locals [3125,6250) at 25000+c*3125+
# (local-3125).  int16 gather indices are handled with two OVERLAPPING table
# windows of this remapped row space:
#   lo table = rows [0, 32768)        idx = row        (max 32767, fits)
#   hi table = rows [17232, 50000)    idx = row-17232  (max 32767, fits)
# Rows in the overlap [17232, 32768) may use either window - preprocess uses
# them to balance each (core, window) bucket's lo count to a multiple of 128,
# removing most chunk-padding waste.
NPIECES = 2                  # independent AllGather pieces per layer
_PSZ = [NPC // NPIECES + (1 if p < NPC % NPIECES else 0) for p in range(NPIECES)]
_POFF = np.cumsum([0] + _PSZ)            # per-core local offsets of pieces
_PBASE = [int(8 * _POFF[p]) for p in range(NPIECES)]   # table base of piece p
TLO_MAX = 32768
THI_BASE = N - 32768         # 17232
NWIN = (NPC + WIN - 1) // WIN            # 196
NGRP = (NWIN + GRP - 1) // GRP           # 13
NLAYERS = 18
SUB = 32                    # max chunks per gather instruction / G tile
FORCE_Q0 = True             # single SWDGE queue measures faster on HW than
                            # alternating queues (and avoids a sim queue-lock)
SINGLE_PACKET = False       # dma_gather single_packet flag
GPOOL_BUFS = 4
SPOOL_BUFS = 4
QSPLIT = False              # split each gather sub-batch across both SWDGE
                            # queues as two concurrent instructions

F32 = mybir.dt.float32
I16 = mybir.dt.int16
MSG_BF16 = False                # message table / gather / selectors in bf16
                                # (bf16 value error ~3e-3/layer amplifies
                                # ~100x over 18 layers -> rel err 0.18; the
                                # 2e-2 gate needs fp32 end to end)
MDT = mybir.dt.bfloat16 if MSG_BF16 else F32
import ml_dtypes
MDT_NP = ml_dtypes.bfloat16 if MSG_BF16 else np.float32


# --------------------------------------------------------------------------
# host-side preprocessing
# --------------------------------------------------------------------------

def preprocess(edge_index, edge_attr):
    src = np.asarray(edge_index[0], dtype=np.int64)
    dst = np.asarray(edge_index[1], dtype=np.int64)
    ew = np.asarray(edge_attr, dtype=np.float32)
    E = src.shape[0]

    owner = dst // NPC
    dst_local = dst - owner * NPC
    win = dst_local // WIN
    # remapped row in the piece-wise allgathered m table: piece p of core c
    # lands at PBASE[p] + c*PSZ[p] + (local - POFF[p])
    src_owner = src // NPC
    src_local = src - src_owner * NPC
    piece = np.searchsorted(_POFF[1:], src_local, side="right")
    psz = np.asarray(_PSZ)[piece]
    pbase = np.asarray(_PBASE)[piece]
    row = pbase + src_owner * psz + (src_local - np.asarray(_POFF)[piece])
    fixed_hi = row >= TLO_MAX
    flex = (row >= THI_BASE) & ~fixed_hi
    half = fixed_hi.astype(np.int64)       # flexible rows start in lo
    # balance: per (owner, win) bucket move flexible rows lo->hi until the lo
    # count is a multiple of 128 (kills per-bucket chunk padding)
    bucket = owner * NWIN + win
    order_b = np.argsort(bucket, kind="stable")
    bs = bucket[order_b]
    b_start = np.searchsorted(bs, np.arange(NCORES * NWIN))
    b_end = np.searchsorted(bs, np.arange(NCORES * NWIN), side="right")
    for b in range(NCORES * NWIN):
        ii = order_b[b_start[b]:b_end[b]]
        k = int((half[ii] == 0).sum() % 128)
        if k:
            fl = ii[flex[ii] & (half[ii] == 0)]
            half[fl[:min(k, len(fl))]] = 1
    tab_idx = np.where(half == 0, row, row - THI_BASE)

    counts = np.zeros((NCORES, NWIN, 2), dtype=np.int64)
    np.add.at(counts, (owner, win, half), 1)
    K = (counts.max(axis=0) + CHUNK - 1) // CHUNK          # [NWIN, 2]
    # Every (window, half) needs >= 1 chunk: each half accumulates in its own
    # PSUM bank and agg = lo + hi, so every column must be written in both.
    K = np.maximum(K, 1)

    order = np.lexsort((half, win, owner))
    so, sw, sh = owner[order], win[order], half[order]
    ssrc, sdl, sew = tab_idx[order], dst_local[order], ew[order]
    starts = {}
    pos = 0
    for c in range(NCORES):
        for w in range(NWIN):
            for h in range(2):
                n = int(counts[c, w, h])
                starts[(c, w, h)] = (pos, pos + n)
                pos += n

    schedule = []           # per group: (n_lo, n_hi, chunk_win list)
    total_chunks = 0
    for g in range(NGRP):
        wlo, whi = g * GRP, min((g + 1) * GRP, NWIN)
        chunk_win = []
        n_lo = n_hi = 0
        for w in range(wlo, whi):
            for _ in range(int(K[w, 0])):
                chunk_win.append(w - wlo)
                n_lo += 1
        for w in range(wlo, whi):
            for _ in range(int(K[w, 1])):
                chunk_win.append(w - wlo)
                n_hi += 1
        schedule.append((n_lo, n_hi, chunk_win))
        total_chunks += n_lo + n_hi

    n_lo_tot = sum(s[0] for s in schedule)
    n_hi_tot = sum(s[1] for s in schedule)

    per_core = []
    for c in range(NCORES):
        lo_idx = np.zeros(max(n_lo_tot, 1) * CHUNK, dtype=np.int16)
        hi_idx = np.zeros(max(n_hi_tot, 1) * CHUNK, dtype=np.int16)
        sel = np.zeros((total_chunks, CHUNK, WIN), dtype=np.float32)
        ci = li = hi_i = 0
        for g in range(NGRP):
            wlo, whi = g * GRP, min((g + 1) * GRP, NWIN)
            for h in (0, 1):
                for w in range(wlo, whi):
                    a, b = starts[(c, w, h)]
                    es, ed, eww = ssrc[a:b], sdl[a:b], sew[a:b]
                    n = b - a
                    for k in range(int(K[w, h])):
                        s0, s1 = k * CHUNK, min((k + 1) * CHUNK, n)
                        cnt = max(0, s1 - s0)
                        if cnt > 0:
                            iv = es[s0:s1].astype(np.int16)
                            if h == 0:
                                lo_idx[li:li + cnt] = iv
                            else:
                                hi_idx[hi_i:hi_i + cnt] = iv
                            sel[ci, np.arange(cnt), ed[s0:s1] - w * WIN] = eww[s0:s1]
                        if h == 0:
                            li += CHUNK
                        else:
                            hi_i += CHUNK
                        ci += 1

        def wrap(flat):
            ncols = len(flat) // 16
            out = np.empty((128, ncols), dtype=np.int16)
            v = flat.reshape(ncols, 16).T
            for g8 in range(8):
                out[g8 * 16:(g8 + 1) * 16] = v
            return out

        per_core.append(dict(
            idx_lo=wrap(lo_idx), idx_hi=wrap(hi_idx),
            sel=np.ascontiguousarray(
                sel.transpose(1, 0, 2).reshape(CHUNK, total_chunks * WIN)
            ).astype(MDT_NP),
        ))
    return schedule, per_core


# --------------------------------------------------------------------------
# program builder
# --------------------------------------------------------------------------

def build_program(schedule, n_layers=NLAYERS, pipeline="m", ablate=""):
    # pipeline=True (AllGathers fired mid-layer, overlapping gathers) measures
    # ~60% SLOWER on HW: the ncfw collective contends with SWDGE gather
    # traffic for the SDMA engines. Keeping the AG pair at the layer boundary
    # (serial, gathers idle) is much faster.
    """ablate: comma-set of {'ag','gather','sel','gru'} to skip (timing only,
    results become wrong)."""
    ab = set(ablate.split(",")) if ablate else set()
    total_chunks = sum(s[0] + s[1] for s in schedule)
    n_lo_tot = sum(s[0] for s in schedule)
    n_hi_tot = sum(s[1] for s in schedule)

    nc = bacc.Bacc("TRN2", target_bir_lowering=False, debug=False,
                   num_devices=NCORES, num_swdge_queues=2)

    xT_in = nc.dram_tensor("xT_in", [128, NPC], F32, kind="ExternalInput")
    idxlo_in = nc.dram_tensor("idx_lo", [128, max(n_lo_tot, 1) * 8], I16, kind="ExternalInput")
    idxhi_in = nc.dram_tensor("idx_hi", [128, max(n_hi_tot, 1) * 8], I16, kind="ExternalInput")
    sel_in = nc.dram_tensor("sel", [128, total_chunks * WIN], MDT, kind="ExternalInput")
    wm_in = nc.dram_tensor("wm", [128, 4 * 128], F32, kind="ExternalInput")
    wg_in = nc.dram_tensor("wg", [128, 12 * 128], F32, kind="ExternalInput")
    gb_in = nc.dram_tensor("gb", [128, 8], F32, kind="ExternalInput")
    outT = nc.dram_tensor("outT", [128, NPC], F32, kind="ExternalOutput")

    # m_own / m_full are double-buffered by layer parity so layer L+1's
    # m-phase + AllGather can run while layer L's gathers still read the
    # previous table.
    m_own = [nc.dram_tensor(f"m_own{i}", [NPC, C], MDT) for i in range(2)]
    m_full = [nc.dram_tensor(f"m_full{i}", [N, C], MDT, addr_space="Shared")
              for i in range(2)]

    NKCH = (NPC + 127) // 128          # 49 node chunks for m-phase

    with tile.TileContext(nc) as tc:
        with (
            tc.tile_pool(name="res", bufs=1) as res,
            tc.tile_pool(name="gpool", bufs=GPOOL_BUFS) as gpool,
            tc.tile_pool(name="spool", bufs=SPOOL_BUFS) as spool,
            tc.tile_pool(name="aggp", bufs=2, space="PSUM") as aggp,
            tc.tile_pool(name="gatep", bufs=5, space="PSUM") as gatep,
            tc.tile_pool(name="mmp", bufs=1, space="PSUM") as mmp,
            tc.tile_pool(name="asb", bufs=2) as asb,
            tc.tile_pool(name="tsb", bufs=10) as tsb,
            tc.tile_pool(name="msb", bufs=4) as msb,
        ):
            # resident tiles
            xT = res.tile([128, NPC], F32)
            idxlo = res.tile([128, max(n_lo_tot, 1) * 8], I16)
            idxhi = res.tile([128, max(n_hi_tot, 1) * 8], I16)
            wm = res.tile([128, 4 * 128], F32)
            wg = res.tile([128, 12 * 128], F32)
            gb = res.tile([128, 8], F32)
            nc.sync.dma_start(xT[:], xT_in[:])
            nc.sync.dma_start(idxlo[:], idxlo_in[:])
            nc.sync.dma_start(idxhi[:], idxhi_in[:])
            nc.sync.dma_start(wm[:], wm_in[:])
            nc.sync.dma_start(wg[:], wg_in[:])
            nc.sync.dma_start(gb[:], gb_in[:])

            def m_chunk(L, k):
                conv = 0 if L < 2 else 1
                wcol = (conv * 2 + (L % 2)) * 128
                c0, c1 = k * 128, min((k + 1) * 128, NPC)
                p = mmp.tile([128, 128], F32, tag="mm")
                nc.tensor.matmul(p[:c1 - c0, :], xT[:, c0:c1],
                                 wm[:, wcol:wcol + 128],
                                 start=True, stop=True)
                s = msb.tile([128, 128], MDT, tag="ms")
                nc.scalar.copy(s[:c1 - c0, :], p[:c1 - c0, :])
                nc.sync.dma_start(m_own[L % 2][c0:c1, :], s[:c1 - c0, :])

            def emit_ag(L):
                if "ag" in ab:
                    return
                # independent piece-AGs issued back-to-back: ncfw overlaps
                # independent collectives (~2x+ vs one chained/big AG)
                for p in range(NPIECES):
                    nc.gpsimd.collective_compute(
                        "AllGather", mybir.AluOpType.bypass,
                        replica_groups=[list(range(NCORES))],
                        ins=[m_own[L % 2][int(_POFF[p]):int(_POFF[p + 1]), :]],
                        outs=[m_full[L % 2][_PBASE[p]:_PBASE[p] + 8 * _PSZ[p], :]],
                    )

            def emit_group(L, g, cursors):
                conv = 0 if L < 2 else 1
                sblk = conv * 6 * 128
                bcol = conv * 4
                relu = (L % 2 == 1) and (L < 17)
                m_lo = m_full[L % 2][0:TLO_MAX, :]
                m_hi = m_full[L % 2][THI_BASE:N, :]

                n_lo, n_hi, chunk_win = schedule[g]
                g0 = g * GRP * WIN
                gw = min(GRP * WIN, NPC - g0)
                # One PSUM bank per half: start=True marks the whole 2KB
                # bank pending-zero, so accumulation groups (windows)
                # must be strictly sequential within a bank.
                agg_lo = aggp.tile([128, 512], F32, tag="agg")
                agg_hi = aggp.tile([128, 512], F32, tag="agg")
                agg2 = [agg_lo, agg_hi]

                # gather + matmul in sub-batches of <= SUB chunks
                j = 0
                while j < n_lo + n_hi:
                    if j < n_lo:
                        nch = min(SUB, n_lo - j)
                        h, idx_t, cur, table = 0, idxlo, cursors["li"], m_lo
                        cursors["li"] += nch
                    else:
                        nch = min(SUB, n_lo + n_hi - j)
                        h, idx_t, cur, table = 1, idxhi, cursors["hi"], m_hi
                        cursors["hi"] += nch
                    h0 = n_lo if h else 0               # half section start
                    h1 = n_lo + n_hi if h else n_lo     # half section end
                    gt = gpool.tile([128, SUB * 128], MDT, tag="g")
                    if "gather" not in ab:
                        if QSPLIT and nch >= 2:
                            nh = (nch + 1) // 2
                            for qn, (a, b) in enumerate(((0, nh), (nh, nch))):
                                nc.gpsimd.dma_gather(
                                    out_ap=gt[:, a * 128:b * 128].rearrange(
                                        "p (a b) -> p a b", b=128),
                                    in_ap=table,
                                    idxs_ap=idx_t[:, (cur + a) * 8:(cur + b) * 8],
                                    num_idxs=(b - a) * 128,
                                    num_idxs_reg=(b - a) * 128,
                                    elem_size=C, single_packet=SINGLE_PACKET,
                                    queue_num=qn,
                                )
                        else:
                            nc.gpsimd.dma_gather(
                                out_ap=gt[:, :nch * 128].rearrange(
                                    "p (a b) -> p a b", b=128),
                                in_ap=table,
                                idxs_ap=idx_t[:, cur * 8:(cur + nch) * 8],
                                num_idxs=nch * 128, num_idxs_reg=nch * 128,
                                elem_size=C, single_packet=SINGLE_PACKET,
                                queue_num=0 if FORCE_Q0 else (cursors["li"] + cursors["hi"]) % 2,
                            )
                    st = spool.tile([128, SUB * WIN], MDT, tag="s")
                    ci = cursors["ci"]
                    nc.sync.dma_start(
                        st[:, :nch * WIN],
                        sel_in[:, (ci + j) * WIN:(ci + j + nch) * WIN])
                    for q in range(0 if "sel" in ab else nch):
                        wg_i = chunk_win[j + q]
                        first = (j + q == h0) or chunk_win[j + q - 1] != wg_i
                        last = (j + q == h1 - 1) or chunk_win[j + q + 1] != wg_i
                        nc.tensor.matmul(
                            agg2[h][:, wg_i * WIN:(wg_i + 1) * WIN],
                            gt[:, q * 128:(q + 1) * 128],
                            st[:, q * WIN:(q + 1) * WIN],
                            start=first, stop=last,
                        )
                    j += nch
                cursors["ci"] += n_lo + n_hi

                # agg^T = lo + hi; a DVE op may read only ONE input from
                # PSUM, so stage hi through the scalar engine first.
                aggs = asb.tile([128, 512], F32, tag="aggs")
                if "sel" in ab:
                    nc.vector.memset(aggs[:, :gw], 0.0)
                else:
                    nc.scalar.copy(aggs[:, :gw], agg2[1][:, :gw])
                    nc.vector.tensor_add(aggs[:, :gw], aggs[:, :gw],
                                         agg2[0][:, :gw])

                # ---- GRU for this 512-node group, feature-major ----
                xg = xT[:, g0:g0 + gw]
                if "gru" in ab:
                    return

                def gate_mm(idx_ih, idx_hh, acc_two):
                    pt = gatep.tile([128, 512], F32, tag="gate")
                    nc.tensor.matmul(
                        pt[:, :gw], wg[:, sblk + idx_ih * 128:sblk + (idx_ih + 1) * 128],
                        aggs[:, :gw], start=True, stop=not acc_two)
                    if acc_two:
                        nc.tensor.matmul(
                            pt[:, :gw], wg[:, sblk + idx_hh * 128:sblk + (idx_hh + 1) * 128],
                            xg, start=False, stop=True)
                    return pt

                r_pre = gate_mm(0, 3, True)          # wihT_r, whhT_r
                z_pre = gate_mm(1, 4, True)
                i_n = gate_mm(2, None, False)        # wihT_n only
                h_n = gatep.tile([128, 512], F32, tag="gate")
                nc.tensor.matmul(h_n[:, :gw], wg[:, sblk + 5 * 128:sblk + 6 * 128],
                                 xg, start=True, stop=True)

                r = tsb.tile([128, 512], F32, tag="t")
                nc.scalar.activation(r[:, :gw], r_pre[:, :gw],
                                     mybir.ActivationFunctionType.Sigmoid,
                                     bias=gb[:, bcol + 0:bcol + 1])
                z = tsb.tile([128, 512], F32, tag="t")
                nc.scalar.activation(z[:, :gw], z_pre[:, :gw],
                                     mybir.ActivationFunctionType.Sigmoid,
                                     bias=gb[:, bcol + 1:bcol + 1 + 1])
                hnb = tsb.tile([128, 512], F32, tag="t")
                nc.vector.tensor_scalar_add(hnb[:, :gw], h_n[:, :gw],
                                            gb[:, bcol + 3:bcol + 4])
                rh = tsb.tile([128, 512], F32, tag="t")
                nc.vector.tensor_mul(rh[:, :gw], r[:, :gw], hnb[:, :gw])
                t1 = tsb.tile([128, 512], F32, tag="t")
                nc.vector.tensor_add(t1[:, :gw], i_n[:, :gw], rh[:, :gw])
                n_t = tsb.tile([128, 512], F32, tag="t")
                nc.scalar.activation(n_t[:, :gw], t1[:, :gw],
                                     mybir.ActivationFunctionType.Tanh,
                                     bias=gb[:, bcol + 2:bcol + 3])
                d = tsb.tile([128, 512], F32, tag="t")
                nc.vector.tensor_sub(d[:, :gw], xg, n_t[:, :gw])
                zd = tsb.tile([128, 512], F32, tag="t")
                nc.vector.tensor_mul(zd[:, :gw], z[:, :gw], d[:, :gw])
                nc.vector.tensor_add(xg, n_t[:, :gw], zd[:, :gw])
                if relu:
                    nc.vector.tensor_scalar_max(xg, xg, 0.0)

            # ---- prologue: layer-0 m-phase + AllGather ----
            for k in range(NKCH):
                m_chunk(0, k)
            emit_ag(0)

            for L in range(n_layers):
                cursors = {"ci": 0, "li": 0, "hi": 0}
                for g in range(NGRP):
                    emit_group(L, g, cursors)
                    if pipeline and L + 1 < n_layers:
                        # layer L+1 m-chunks for the nodes this group just
                        # updated; the AllGather stays at the layer boundary
                        # (overlapping it with gathers contends on SDMA).
                        for k in range(4 * g, min(4 * g + 4, NKCH)):
                            m_chunk(L + 1, k)
                if L + 1 < n_layers:
                    if not pipeline:
                        for k in range(NKCH):
                            m_chunk(L + 1, k)
                    emit_ag(L + 1)

            nc.sync.dma_start(outT[:], xT[:])

    nc.compile()
    return nc


# --------------------------------------------------------------------------
# entry point
# --------------------------------------------------------------------------

def _pack_params(inputs):
    wm = np.zeros((128, 4 * 128), dtype=np.float32)
    wg = np.zeros((128, 12 * 128), dtype=np.float32)
    gb = np.zeros((128, 8), dtype=np.float32)
    for conv, tag in ((0, "1"), (1, "2")):
        w = np.asarray(inputs[f"w{tag}"], dtype=np.float32)
        wih = np.asarray(inputs[f"wih{tag}"], dtype=np.float32)
        whh = np.asarray(inputs[f"whh{tag}"], dtype=np.float32)
        bih = np.asarray(inputs[f"bih{tag}"], dtype=np.float32)
        bhh = np.asarray(inputs[f"bhh{tag}"], dtype=np.float32)
        for l in range(2):
            wm[:, (conv * 2 + l) * 128:(conv * 2 + l + 1) * 128] = w[l]
        for i, mat in enumerate((wih[0:128], wih[128:256], wih[256:384],
                                 whh[0:128], whh[128:256], whh[256:384])):
            wg[:, (conv * 6 + i) * 128:(conv * 6 + i + 1) * 128] = mat.T
        gb[:, conv * 4 + 0] = bih[0:128] + bhh[0:128]
        gb[:, conv * 4 + 1] = bih[128:256] + bhh[128:256]
        gb[:, conv * 4 + 2] = bih[256:384]
        gb[:, conv * 4 + 3] = bhh[256:384]
    return wm, wg, gb


_CACHE = {}


def kernel(**inputs):
    x = np.asarray(inputs["x"], dtype=np.float32)
    schedule, per_core = preprocess(inputs["edge_index"], inputs["edge_attr"])
    wm, wg, gb = _pack_params(inputs)

    key = tuple((s[0], s[1]) for s in schedule)
    if key not in _CACHE:
        _CACHE[key] = build_program(schedule)
    nc = _CACHE[key]

    in_maps = []
    for c in range(NCORES):
        pc = per_core[c]
        in_maps.append({
            "xT_in": np.ascontiguousarray(x[c * NPC:(c + 1) * NPC].T),
            "idx_lo": pc["idx_lo"], "idx_hi": pc["idx_hi"],
            "sel": pc["sel"], "wm": wm, "wg": wg, "gb": gb,
        })
    res = bass_utils.run_bass_kernel_spmd(nc, in_maps, list(range(NCORES)))
    out = np.concatenate(
        [res.results[c]["outT"].T for c in range(NCORES)], axis=0)
    return out.astype(np.float32)



# revision 42
# speedup vs baseline: 1.2020x; 1.2020x over previous
"""Trainium2 Bass kernel for a 2-layer GatedGraphConv encoder (9 convs, 18
message-passing + GRU steps) on N=50000 nodes, E=800000 edges, C=128.

Strategy (8 NeuronCores, SPMD single program):
  - Nodes are block-sharded: core c owns dst rows [c*6250, (c+1)*6250).
  - Edges live on the core that owns their dst.  Per core, edges are bucketed
    into 32-dst windows and padded to 128-edge chunks (chunk counts are the
    max over cores so the shared program works for every core).
  - Per layer: each core computes its m = x @ W slice with the tensor engine,
    an AllGather materializes the full message table in DRAM, dma_gather
    pulls the per-edge message rows into SBUF, and one 128x32 matmul per
    chunk (selector = ew-scaled one-hot of dst-in-window) accumulates
    agg^T = sum_e ew_e * m[src_e] directly in PSUM, feature-major.
  - The GRU update runs entirely feature-major: gate matmuls stream the
    512-node PSUM group through pre-transposed GRU weights; sigmoids/tanh on
    the scalar engine (per-partition bias = per-channel bias), elementwise on
    the vector engine.  x^T stays resident in SBUF across all 18 layers.
"""
import numpy as np

import concourse.bacc as bacc
import concourse.mybir as mybir
import concourse.tile as tile
from concourse import bass_utils

N = 50000
C = 128
NCORES = 8
NPC = N // NCORES            # 6250
WIN = 64                     # dst nodes per selector window
GRP = 8                      # windows per 512-col PSUM group
CHUNK = 128                  # edges per selector matmul
# Two INDEPENDENT half-AllGathers per layer into m_full [50000, 128] (HW
# overlaps independent collectives ~2x vs chained).  Core c's local nodes
# [0,3125) land at rows c*3125+local, locals [3125,6250) at 25000+c*3125+
# (local-3125).  int16 gather indices are handled with two OVERLAPPING table
# windows of this remapped row space:
#   lo table = rows [0, 32768)        idx = row        (max 32767, fits)
#   hi table = rows [17232, 50000)    idx = row-17232  (max 32767, fits)
# Rows in the overlap [17232, 32768) may use either window - preprocess uses
# them to balance each (core, window) bucket's lo count to a multiple of 128,
# removing most chunk-padding waste.
NPIECES = 2                  # independent AllGather pieces per layer
_PSZ = [NPC // NPIECES + (1 if p < NPC % NPIECES else 0) for p in range(NPIECES)]
_POFF = np.cumsum([0] + _PSZ)            # per-core local offsets of pieces
_PBASE = [int(8 * _POFF[p]) for p in range(NPIECES)]   # table base of piece p
TLO_MAX = 32768
THI_BASE = N - 32768         # 17232
NWIN = (NPC + WIN - 1) // WIN            # 196
NGRP = (NWIN + GRP - 1) // GRP           # 13
NLAYERS = 18
SUB = 32                    # max chunks per gather instruction / G tile
FORCE_Q0 = True             # single SWDGE queue measures faster on HW than
                            # alternating queues (and avoids a sim queue-lock)
SINGLE_PACKET = False       # dma_gather single_packet flag
GPOOL_BUFS = 4
SPOOL_BUFS = 4
QSPLIT = False              # split each gather sub-batch across both SWDGE
                            # queues as two concurrent instructions
DMA_SCRATCH = 16384         # SWDGE descriptor-ring carveout bytes (1024 descs)

F32 = mybir.dt.float32
I16 = mybir.dt.int16
MSG_BF16 = False                # message table / gather / selectors in bf16
                                # (bf16 value error ~3e-3/layer amplifies
                                # ~100x over 18 layers -> rel err 0.18; the
                                # 2e-2 gate needs fp32 end to end)
MDT = mybir.dt.bfloat16 if MSG_BF16 else F32
import ml_dtypes
MDT_NP = ml_dtypes.bfloat16 if MSG_BF16 else np.float32


# --------------------------------------------------------------------------
# host-side preprocessing
# --------------------------------------------------------------------------

def preprocess(edge_index, edge_attr):
    src = np.asarray(edge_index[0], dtype=np.int64)
    dst = np.asarray(edge_index[1], dtype=np.int64)
    ew = np.asarray(edge_attr, dtype=np.float32)
    E = src.shape[0]

    owner = dst // NPC
    dst_local = dst - owner * NPC
    win = dst_local // WIN
    # remapped row in the piece-wise allgathered m table: piece p of core c
    # lands at PBASE[p] + c*PSZ[p] + (local - POFF[p])
    src_owner = src // NPC
    src_local = src - src_owner * NPC
    piece = np.searchsorted(_POFF[1:], src_local, side="right")
    psz = np.asarray(_PSZ)[piece]
    pbase = np.asarray(_PBASE)[piece]
    row = pbase + src_owner * psz + (src_local - np.asarray(_POFF)[piece])
    fixed_hi = row >= TLO_MAX
    flex = (row >= THI_BASE) & ~fixed_hi
    half = fixed_hi.astype(np.int64)       # flexible rows start in lo
    # balance: per (owner, win) bucket move flexible rows lo->hi until the lo
    # count is a multiple of 128 (kills per-bucket chunk padding)
    bucket = owner * NWIN + win
    order_b = np.argsort(bucket, kind="stable")
    bs = bucket[order_b]
    b_start = np.searchsorted(bs, np.arange(NCORES * NWIN))
    b_end = np.searchsorted(bs, np.arange(NCORES * NWIN), side="right")
    for b in range(NCORES * NWIN):
        ii = order_b[b_start[b]:b_end[b]]
        k = int((half[ii] == 0).sum() % 128)
        if k:
            fl = ii[flex[ii] & (half[ii] == 0)]
            half[fl[:min(k, len(fl))]] = 1
    tab_idx = np.where(half == 0, row, row - THI_BASE)

    counts = np.zeros((NCORES, NWIN, 2), dtype=np.int64)
    np.add.at(counts, (owner, win, half), 1)
    K = (counts.max(axis=0) + CHUNK - 1) // CHUNK          # [NWIN, 2]
    # Every (window, half) needs >= 1 chunk: each half accumulates in its own
    # PSUM bank and agg = lo + hi, so every column must be written in both.
    K = np.maximum(K, 1)

    order = np.lexsort((half, win, owner))
    so, sw, sh = owner[order], win[order], half[order]
    ssrc, sdl, sew = tab_idx[order], dst_local[order], ew[order]
    starts = {}
    pos = 0
    for c in range(NCORES):
        for w in range(NWIN):
            for h in range(2):
                n = int(counts[c, w, h])
                starts[(c, w, h)] = (pos, pos + n)
                pos += n

    schedule = []           # per group: (n_lo, n_hi, chunk_win list)
    total_chunks = 0
    for g in range(NGRP):
        wlo, whi = g * GRP, min((g + 1) * GRP, NWIN)
        chunk_win = []
        n_lo = n_hi = 0
        for w in range(wlo, whi):
            for _ in range(int(K[w, 0])):
                chunk_win.append(w - wlo)
                n_lo += 1
        for w in range(wlo, whi):
            for _ in range(int(K[w, 1])):
                chunk_win.append(w - wlo)
                n_hi += 1
        schedule.append((n_lo, n_hi, chunk_win))
        total_chunks += n_lo + n_hi

    n_lo_tot = sum(s[0] for s in schedule)
    n_hi_tot = sum(s[1] for s in schedule)

    per_core = []
    for c in range(NCORES):
        lo_idx = np.zeros(max(n_lo_tot, 1) * CHUNK, dtype=np.int16)
        hi_idx = np.zeros(max(n_hi_tot, 1) * CHUNK, dtype=np.int16)
        sel = np.zeros((total_chunks, CHUNK, WIN), dtype=np.float32)
        ci = li = hi_i = 0
        for g in range(NGRP):
            wlo, whi = g * GRP, min((g + 1) * GRP, NWIN)
            for h in (0, 1):
                for w in range(wlo, whi):
                    a, b = starts[(c, w, h)]
                    es, ed, eww = ssrc[a:b], sdl[a:b], sew[a:b]
                    n = b - a
                    for k in range(int(K[w, h])):
                        s0, s1 = k * CHUNK, min((k + 1) * CHUNK, n)
                        cnt = max(0, s1 - s0)
                        if cnt > 0:
                            iv = es[s0:s1].astype(np.int16)
                            if h == 0:
                                lo_idx[li:li + cnt] = iv
                            else:
                                hi_idx[hi_i:hi_i + cnt] = iv
                            sel[ci, np.arange(cnt), ed[s0:s1] - w * WIN] = eww[s0:s1]
                        if h == 0:
                            li += CHUNK
                        else:
                            hi_i += CHUNK
                        ci += 1

        def wrap(flat):
            ncols = len(flat) // 16
            out = np.empty((128, ncols), dtype=np.int16)
            v = flat.reshape(ncols, 16).T
            for g8 in range(8):
                out[g8 * 16:(g8 + 1) * 16] = v
            return out

        per_core.append(dict(
            idx_lo=wrap(lo_idx), idx_hi=wrap(hi_idx),
            sel=np.ascontiguousarray(
                sel.transpose(1, 0, 2).reshape(CHUNK, total_chunks * WIN)
            ).astype(MDT_NP),
        ))
    return schedule, per_core


# --------------------------------------------------------------------------
# program builder
# --------------------------------------------------------------------------

def build_program(schedule, n_layers=NLAYERS, pipeline="m", ablate=""):
    # pipeline=True (AllGathers fired mid-layer, overlapping gathers) measures
    # ~60% SLOWER on HW: the ncfw collective contends with SWDGE gather
    # traffic for the SDMA engines. Keeping the AG pair at the layer boundary
    # (serial, gathers idle) is much faster.
    """ablate: comma-set of {'ag','gather','sel','gru'} to skip (timing only,
    results become wrong)."""
    ab = set(ablate.split(",")) if ablate else set()
    total_chunks = sum(s[0] + s[1] for s in schedule)
    n_lo_tot = sum(s[0] for s in schedule)
    n_hi_tot = sum(s[1] for s in schedule)

    nc = bacc.Bacc("TRN2", target_bir_lowering=False, debug=False,
                   num_devices=NCORES, num_swdge_queues=2,
                   dynamic_dma_scratch_size=DMA_SCRATCH)

    xT_in = nc.dram_tensor("xT_in", [128, NPC], F32, kind="ExternalInput")
    idxlo_in = nc.dram_tensor("idx_lo", [128, max(n_lo_tot, 1) * 8], I16, kind="ExternalInput")
    idxhi_in = nc.dram_tensor("idx_hi", [128, max(n_hi_tot, 1) * 8], I16, kind="ExternalInput")
    sel_in = nc.dram_tensor("sel", [128, total_chunks * WIN], MDT, kind="ExternalInput")
    wm_in = nc.dram_tensor("wm", [128, 4 * 128], F32, kind="ExternalInput")
    wg_in = nc.dram_tensor("wg", [128, 12 * 128], F32, kind="ExternalInput")
    gb_in = nc.dram_tensor("gb", [128, 8], F32, kind="ExternalInput")
    outT = nc.dram_tensor("outT", [128, NPC], F32, kind="ExternalOutput")

    # m_own / m_full are double-buffered by layer parity so layer L+1's
    # m-phase + AllGather can run while layer L's gathers still read the
    # previous table.
    m_own = [nc.dram_tensor(f"m_own{i}", [NPC, C], MDT) for i in range(2)]
    m_full = [nc.dram_tensor(f"m_full{i}", [N, C], MDT, addr_space="Shared")
              for i in range(2)]

    NKCH = (NPC + 127) // 128          # 49 node chunks for m-phase

    with tile.TileContext(nc) as tc:
        with (
            tc.tile_pool(name="res", bufs=1) as res,
            tc.tile_pool(name="gpool", bufs=GPOOL_BUFS) as gpool,
            tc.tile_pool(name="spool", bufs=SPOOL_BUFS) as spool,
            tc.tile_pool(name="aggp", bufs=2, space="PSUM") as aggp,
            tc.tile_pool(name="gatep", bufs=5, space="PSUM") as gatep,
            tc.tile_pool(name="mmp", bufs=1, space="PSUM") as mmp,
            tc.tile_pool(name="asb", bufs=2) as asb,
            tc.tile_pool(name="tsb", bufs=10) as tsb,
            tc.tile_pool(name="msb", bufs=4) as msb,
        ):
            # resident tiles
            xT = res.tile([128, NPC], F32)
            idxlo = res.tile([128, max(n_lo_tot, 1) * 8], I16)
            idxhi = res.tile([128, max(n_hi_tot, 1) * 8], I16)
            wm = res.tile([128, 4 * 128], F32)
            wg = res.tile([128, 12 * 128], F32)
            gb = res.tile([128, 8], F32)
            nc.sync.dma_start(xT[:], xT_in[:])
            nc.sync.dma_start(idxlo[:], idxlo_in[:])
            nc.sync.dma_start(idxhi[:], idxhi_in[:])
            nc.sync.dma_start(wm[:], wm_in[:])
            nc.sync.dma_start(wg[:], wg_in[:])
            nc.sync.dma_start(gb[:], gb_in[:])

            def m_chunk(L, k):
                conv = 0 if L < 2 else 1
                wcol = (conv * 2 + (L % 2)) * 128
                c0, c1 = k * 128, min((k + 1) * 128, NPC)
                p = mmp.tile([128, 128], F32, tag="mm")
                nc.tensor.matmul(p[:c1 - c0, :], xT[:, c0:c1],
                                 wm[:, wcol:wcol + 128],
                                 start=True, stop=True)
                s = msb.tile([128, 128], MDT, tag="ms")
                nc.scalar.copy(s[:c1 - c0, :], p[:c1 - c0, :])
                nc.sync.dma_start(m_own[L % 2][c0:c1, :], s[:c1 - c0, :])

            def emit_ag(L):
                if "ag" in ab:
                    return
                # independent piece-AGs issued back-to-back: ncfw overlaps
                # independent collectives (~2x+ vs one chained/big AG)
                for p in range(NPIECES):
                    nc.gpsimd.collective_compute(
                        "AllGather", mybir.AluOpType.bypass,
                        replica_groups=[list(range(NCORES))],
                        ins=[m_own[L % 2][int(_POFF[p]):int(_POFF[p + 1]), :]],
                        outs=[m_full[L % 2][_PBASE[p]:_PBASE[p] + 8 * _PSZ[p], :]],
                    )

            def emit_group(L, g, cursors):
                conv = 0 if L < 2 else 1
                sblk = conv * 6 * 128
                bcol = conv * 4
                relu = (L % 2 == 1) and (L < 17)
                m_lo = m_full[L % 2][0:TLO_MAX, :]
                m_hi = m_full[L % 2][THI_BASE:N, :]

                n_lo, n_hi, chunk_win = schedule[g]
                g0 = g * GRP * WIN
                gw = min(GRP * WIN, NPC - g0)
                # One PSUM bank per half: start=True marks the whole 2KB
                # bank pending-zero, so accumulation groups (windows)
                # must be strictly sequential within a bank.
                agg_lo = aggp.tile([128, 512], F32, tag="agg")
                agg_hi = aggp.tile([128, 512], F32, tag="agg")
                agg2 = [agg_lo, agg_hi]

                # gather + matmul in sub-batches of <= SUB chunks
                j = 0
                while j < n_lo + n_hi:
                    if j < n_lo:
                        nch = min(SUB, n_lo - j)
                        h, idx_t, cur, table = 0, idxlo, cursors["li"], m_lo
                        cursors["li"] += nch
                    else:
                        nch = min(SUB, n_lo + n_hi - j)
                        h, idx_t, cur, table = 1, idxhi, cursors["hi"], m_hi
                        cursors["hi"] += nch
                    h0 = n_lo if h else 0               # half section start
                    h1 = n_lo + n_hi if h else n_lo     # half section end
                    gt = gpool.tile([128, SUB * 128], MDT, tag="g")
                    if "gather" not in ab:
                        if QSPLIT and nch >= 2:
                            nh = (nch + 1) // 2
                            for qn, (a, b) in enumerate(((0, nh), (nh, nch))):
                                nc.gpsimd.dma_gather(
                                    out_ap=gt[:, a * 128:b * 128].rearrange(
                                        "p (a b) -> p a b", b=128),
                                    in_ap=table,
                                    idxs_ap=idx_t[:, (cur + a) * 8:(cur + b) * 8],
                                    num_idxs=(b - a) * 128,
                                    num_idxs_reg=(b - a) * 128,
                                    elem_size=C, single_packet=SINGLE_PACKET,
                                    queue_num=qn,
                                )
                        else:
                            nc.gpsimd.dma_gather(
                                out_ap=gt[:, :nch * 128].rearrange(
                                    "p (a b) -> p a b", b=128),
                                in_ap=table,
                                idxs_ap=idx_t[:, cur * 8:(cur + nch) * 8],
                                num_idxs=nch * 128, num_idxs_reg=nch * 128,
                                elem_size=C, single_packet=SINGLE_PACKET,
                                queue_num=0 if FORCE_Q0 else (cursors["li"] + cursors["hi"]) % 2,
                            )
                    st = spool.tile([128, SUB * WIN], MDT, tag="s")
                    ci = cursors["ci"]
                    nc.sync.dma_start(
                        st[:, :nch * WIN],
                        sel_in[:, (ci + j) * WIN:(ci + j + nch) * WIN])
                    for q in range(0 if "sel" in ab else nch):
                        wg_i = chunk_win[j + q]
                        first = (j + q == h0) or chunk_win[j + q - 1] != wg_i
                        last = (j + q == h1 - 1) or chunk_win[j + q + 1] != wg_i
                        nc.tensor.matmul(
                            agg2[h][:, wg_i * WIN:(wg_i + 1) * WIN],
                            gt[:, q * 128:(q + 1) * 128],
                            st[:, q * WIN:(q + 1) * WIN],
                            start=first, stop=last,
                        )
                    j += nch
                cursors["ci"] += n_lo + n_hi

                # agg^T = lo + hi; a DVE op may read only ONE input from
                # PSUM, so stage hi through the scalar engine first.
                aggs = asb.tile([128, 512], F32, tag="aggs")
                if "sel" in ab:
                    nc.vector.memset(aggs[:, :gw], 0.0)
                else:
                    nc.scalar.copy(aggs[:, :gw], agg2[1][:, :gw])
                    nc.vector.tensor_add(aggs[:, :gw], aggs[:, :gw],
                                         agg2[0][:, :gw])

                # ---- GRU for this 512-node group, feature-major ----
                xg = xT[:, g0:g0 + gw]
                if "gru" in ab:
                    return

                def gate_mm(idx_ih, idx_hh, acc_two):
                    pt = gatep.tile([128, 512], F32, tag="gate")
                    nc.tensor.matmul(
                        pt[:, :gw], wg[:, sblk + idx_ih * 128:sblk + (idx_ih + 1) * 128],
                        aggs[:, :gw], start=True, stop=not acc_two)
                    if acc_two:
                        nc.tensor.matmul(
                            pt[:, :gw], wg[:, sblk + idx_hh * 128:sblk + (idx_hh + 1) * 128],
                            xg, start=False, stop=True)
                    return pt

                r_pre = gate_mm(0, 3, True)          # wihT_r, whhT_r
                z_pre = gate_mm(1, 4, True)
                i_n = gate_mm(2, None, False)        # wihT_n only
                h_n = gatep.tile([128, 512], F32, tag="gate")
                nc.tensor.matmul(h_n[:, :gw], wg[:, sblk + 5 * 128:sblk + 6 * 128],
                                 xg, start=True, stop=True)

                r = tsb.tile([128, 512], F32, tag="t")
                nc.scalar.activation(r[:, :gw], r_pre[:, :gw],
                                     mybir.ActivationFunctionType.Sigmoid,
                                     bias=gb[:, bcol + 0:bcol + 1])
                z = tsb.tile([128, 512], F32, tag="t")
                nc.scalar.activation(z[:, :gw], z_pre[:, :gw],
                                     mybir.ActivationFunctionType.Sigmoid,
                                     bias=gb[:, bcol + 1:bcol + 1 + 1])
                hnb = tsb.tile([128, 512], F32, tag="t")
                nc.vector.tensor_scalar_add(hnb[:, :gw], h_n[:, :gw],
                                            gb[:, bcol + 3:bcol + 4])
                rh = tsb.tile([128, 512], F32, tag="t")
                nc.vector.tensor_mul(rh[:, :gw], r[:, :gw], hnb[:, :gw])
                t1 = tsb.tile([128, 512], F32, tag="t")
                nc.vector.tensor_add(t1[:, :gw], i_n[:, :gw], rh[:, :gw])
                n_t = tsb.tile([128, 512], F32, tag="t")
                nc.scalar.activation(n_t[:, :gw], t1[:, :gw],
                                     mybir.ActivationFunctionType.Tanh,
                                     bias=gb[:, bcol + 2:bcol + 3])
                d = tsb.tile([128, 512], F32, tag="t")
                nc.vector.tensor_sub(d[:, :gw], xg, n_t[:, :gw])
                zd = tsb.tile([128, 512], F32, tag="t")
                nc.vector.tensor_mul(zd[:, :gw], z[:, :gw], d[:, :gw])
                nc.vector.tensor_add(xg, n_t[:, :gw], zd[:, :gw])
                if relu:
                    nc.vector.tensor_scalar_max(xg, xg, 0.0)

            # ---- prologue: layer-0 m-phase + AllGather ----
            for k in range(NKCH):
                m_chunk(0, k)
            emit_ag(0)

            for L in range(n_layers):
                cursors = {"ci": 0, "li": 0, "hi": 0}
                for g in range(NGRP):
                    emit_group(L, g, cursors)
                    if pipeline and L + 1 < n_layers:
                        # layer L+1 m-chunks for the nodes this group just
                        # updated; the AllGather stays at the layer boundary
                        # (overlapping it with gathers contends on SDMA).
                        for k in range(4 * g, min(4 * g + 4, NKCH)):
                            m_chunk(L + 1, k)
                if L + 1 < n_layers:
                    if not pipeline:
                        for k in range(NKCH):
                            m_chunk(L + 1, k)
                    emit_ag(L + 1)

            nc.sync.dma_start(outT[:], xT[:])

    nc.compile()
    return nc


# --------------------------------------------------------------------------
# entry point
# --------------------------------------------------------------------------

def _pack_params(inputs):
    wm = np.zeros((128, 4 * 128), dtype=np.float32)
    wg = np.zeros((128, 12 * 128), dtype=np.float32)
    gb = np.zeros((128, 8), dtype=np.float32)
    for conv, tag in ((0, "1"), (1, "2")):
        w = np.asarray(inputs[f"w{tag}"], dtype=np.float32)
        wih = np.asarray(inputs[f"wih{tag}"], dtype=np.float32)
        whh = np.asarray(inputs[f"whh{tag}"], dtype=np.float32)
        bih = np.asarray(inputs[f"bih{tag}"], dtype=np.float32)
        bhh = np.asarray(inputs[f"bhh{tag}"], dtype=np.float32)
        for l in range(2):
            wm[:, (conv * 2 + l) * 128:(conv * 2 + l + 1) * 128] = w[l]
        for i, mat in enumerate((wih[0:128], wih[128:256], wih[256:384],
                                 whh[0:128], whh[128:256], whh[256:384])):
            wg[:, (conv * 6 + i) * 128:(conv * 6 + i + 1) * 128] = mat.T
        gb[:, conv * 4 + 0] = bih[0:128] + bhh[0:128]
        gb[:, conv * 4 + 1] = bih[128:256] + bhh[128:256]
        gb[:, conv * 4 + 2] = bih[256:384]
        gb[:, conv * 4 + 3] = bhh[256:384]
    return wm, wg, gb


_CACHE = {}


def kernel(**inputs):
    x = np.asarray(inputs["x"], dtype=np.float32)
    schedule, per_core = preprocess(inputs["edge_index"], inputs["edge_attr"])
    wm, wg, gb = _pack_params(inputs)

    key = tuple((s[0], s[1]) for s in schedule)
    if key not in _CACHE:
        _CACHE[key] = build_program(schedule)
    nc = _CACHE[key]

    in_maps = []
    for c in range(NCORES):
        pc = per_core[c]
        in_maps.append({
            "xT_in": np.ascontiguousarray(x[c * NPC:(c + 1) * NPC].T),
            "idx_lo": pc["idx_lo"], "idx_hi": pc["idx_hi"],
            "sel": pc["sel"], "wm": wm, "wg": wg, "gb": gb,
        })
    res = bass_utils.run_bass_kernel_spmd(nc, in_maps, list(range(NCORES)))
    out = np.concatenate(
        [res.results[c]["outT"].T for c in range(NCORES)], axis=0)
    return out.astype(np.float32)

